# revision 1
# baseline (speedup 1.0000x reference)
"""DFlashAttention (paged KV cache decode-attention block) on 8 Trainium2
NeuronCores.

Sharding: tensor-parallel over heads. Each core owns HQ/8 = 4 query heads and
HK/8 = 1 KV head (GQA group). Wq/Wk/Wv row-sharded, Wo column-sharded; each
core produces a partial output [B*S, HID] (stored bf16) which is summed on
the host.

Device kernel layout choices (v2, bf16 data path):
  - All matmul operands in bfloat16 (same PE rate as f32r for >=256-col
    moving operands, half the DMA bytes); PSUM accumulation stays f32.
  - Projections produce q/k/v in [D, token] layout (head dim on partitions).
  - Scores are computed transposed: [l_chunk(128), (head, s)=512] with the
    KV-cache chunk as the stationary operand, so PV consumes probs directly.
  - Softmax-sum and RMS-norm sums use an all-ones [128,128] stationary, so
    the per-column sums land in PSUM already broadcast across partitions:
    normalization is then a plain elementwise multiply - no broadcast
    matmuls, no [1,N] lane-wasting ops.
  - sin/cos evaluated on the host (f64) and shipped as bf16 tables; no
    activation-table load for Sin on device.
  - kT/vC caches and Wo are loaded with one large DMA each and stay resident
    for the rep; the fresh-V [d,s]->[s,d] flip uses dma_start_transpose
    (XBAR) instead of PE transposes.
  - Per-batch cache lengths are baked into the instruction stream at build
    time; the final partial cache chunk is masked by accumulating a rank-1
    (-1e30) outer product into the scores so exp underflows to exactly zero.
  - Output projection for batch b is emitted inside batch b+1's attention so
    its matmuls fill PE gaps; batches are processed in descending cache
    length order.
"""

import sys

sys.path.insert(0, "/opt/trn_rl_repo")

import numpy as np

B, S, HID = 4, 128, 4096
D, HQ, HK = 128, 32, 8
PAGES, PSIZE, NPP = 64, 256, 16
THETA = 10000.0
EPS = 1e-6
N_CORES = 8
HQC = HQ // N_CORES  # 4 query heads per core
EC = HQC * D         # 512 output-proj contraction per core
BS = B * S           # 512 tokens
NDCH = HID // 128    # 32 contraction chunks for projections

_CACHE = {}


def _prep_host(x, Wq, Wk, Wv, Wo, q_norm_w, k_norm_w, k_cache, v_cache,
               block_table, cache_seqlens):
    import ml_dtypes
    BF = ml_dtypes.bfloat16
    f32 = np.float32

    xT = np.ascontiguousarray(
        np.asarray(x, f32).reshape(BS, HID).T).astype(BF)

    lens = [int(v) for v in np.asarray(cache_seqlens)]
    pads = [(l + 127) // 128 * 128 for l in lens]
    offs = [0] * B
    for b in range(1, B):
        offs[b] = offs[b - 1] + pads[b - 1]
    total = max(sum(pads), 128)

    bt = np.asarray(block_table)
    kg = np.asarray(k_cache, f32)[bt].reshape(B, NPP * PSIZE, HK, D)
    vg = np.asarray(v_cache, f32)[bt].reshape(B, NPP * PSIZE, HK, D)

    # RoPE sin/cos evaluated on host in f64 on the reference's fp32 freqs.
    pos = np.asarray(cache_seqlens, np.float64)[:, None] + np.arange(S)[None, :]
    inv = 1.0 / (THETA ** (np.arange(0, D, 2, dtype=np.float64) / D))
    freqs32 = (pos.astype(f32)[:, :, None]
               * inv.astype(f32)[None, None, :]).astype(f32)
    fr = np.float64(freqs32)
    sin_h = np.sin(fr).reshape(BS, 64).T            # [64, BS]
    cos_h = np.cos(fr).reshape(BS, 64).T
    sin2 = np.concatenate([sin_h, sin_h], 0)        # [128, BS]
    cos2 = np.concatenate([cos_h, cos_h], 0)
    # doubled along columns so one op covers a 2-head [128, 2*BS] tile
    sct = np.ascontiguousarray(np.concatenate(
        [sin2, sin2, cos2, cos2], 1)).astype(BF)    # [128, 4*BS]

    # f32 per-partition scalars: wqA wqB wkA wkB epsq epsk
    wq_ = np.asarray(q_norm_w, f32).reshape(D)
    wk_ = np.asarray(k_norm_w, f32).reshape(D)
    cf = np.stack([wq_, np.roll(wq_, 64), wk_, np.roll(wk_, 64),
                   np.full(D, D * EPS, f32), np.full(D, EPS, f32)], 1)

    # bf16 consts: [allones(128) | masks(4*128) | negrow(512) | wqA wqB
    # wkA wkB (4 per-partition scalar cols)]
    cb = np.zeros((128, 1156), f32)
    cb[:, 0:128] = 1.0
    for b in range(B):
        t = lens[b] - (pads[b] // 128 - 1) * 128 if pads[b] > 0 else 128
        cb[0, 128 + b * 128 + t:128 + (b + 1) * 128] = 1.0
    cb[0, 640:1152] = -1e30
    cb[:, 1152] = wq_
    cb[:, 1153] = np.roll(wq_, 64)
    cb[:, 1154] = wk_
    cb[:, 1155] = np.roll(wk_, 64)
    cb = cb.astype(BF)

    Wq_ = np.asarray(Wq, f32)
    Wk_ = np.asarray(Wk, f32)
    Wv_ = np.asarray(Wv, f32)
    Wo_ = np.asarray(Wo, f32)

    in_maps = []
    for c in range(N_CORES):
        wqT = np.ascontiguousarray(Wq_[c * EC:(c + 1) * EC, :].T).astype(BF)
        wkvT = np.ascontiguousarray(
            np.concatenate([Wk_[c * D:(c + 1) * D, :],
                            Wv_[c * D:(c + 1) * D, :]], 0).T).astype(BF)
        woT = np.ascontiguousarray(Wo_[:, c * EC:(c + 1) * EC].T).astype(BF)
        kT = np.zeros((128, total), f32)
        vCf = np.zeros((total, 128), f32)
        for b in range(B):
            nb, ob = lens[b], offs[b]
            if nb > 0:
                kT[:, ob:ob + nb] = kg[b, :nb, c, :].T
                vCf[ob:ob + nb, :] = vg[b, :nb, c, :]
        vP = np.ascontiguousarray(
            vCf.reshape(total // 128, 128, 128).transpose(1, 0, 2)
            .reshape(128, total))
        in_maps.append(dict(
            xT=xT, wqT=wqT, wkvT=wkvT, woT=woT,
            kT=np.ascontiguousarray(kT).astype(BF), vC=vP.astype(BF),
            sct=sct, cf=np.ascontiguousarray(cf), cb=cb,
        ))
    return in_maps, lens, pads, offs, total


def _build_nc(lens, pads, offs, total, reps=1):
    import concourse.mybir as mybir
    import concourse.tile as tile
    from concourse import bacc

    F32 = mybir.dt.float32
    BF16 = mybir.dt.bfloat16
    AF = mybir.ActivationFunctionType
    OP = mybir.AluOpType

    nc = bacc.Bacc("TRN2", target_bir_lowering=False, debug=False,
                   num_devices=N_CORES)

    xT_d = nc.dram_tensor("xT", [HID, BS], BF16, kind="ExternalInput")
    wqT_d = nc.dram_tensor("wqT", [HID, EC], BF16, kind="ExternalInput")
    wkvT_d = nc.dram_tensor("wkvT", [HID, 2 * D], BF16, kind="ExternalInput")
    woT_d = nc.dram_tensor("woT", [EC, HID], BF16, kind="ExternalInput")
    kT_d = nc.dram_tensor("kT", [128, total], BF16, kind="ExternalInput")
    vC_d = nc.dram_tensor("vC", [128, total], BF16, kind="ExternalInput")
    sct_d = nc.dram_tensor("sct", [128, 4 * BS], BF16, kind="ExternalInput")
    cf_d = nc.dram_tensor("cf", [128, 6], F32, kind="ExternalInput")
    cb_d = nc.dram_tensor("cb", [128, 1156], BF16, kind="ExternalInput")
    out_d = nc.dram_tensor("out", [BS, HID], BF16, kind="ExternalOutput")

    xT_v = xT_d.rearrange("(c p) e -> p c e", p=128)     # [128, 32, 512]
    wqT_v = wqT_d.rearrange("(c p) e -> p c e", p=128)   # [128, 32, 512]
    wkvT_v = wkvT_d.rearrange("(c p) e -> p c e", p=128) # [128, 32, 256]
    woT_v = woT_d.rearrange("(c p) e -> p c e", p=128)   # [128, 4, 4096]

    nch = [pads[b] // 128 for b in range(B)]
    border = sorted(range(B), key=lambda b: -nch[b])

    with tile.TileContext(nc) as tc:
        with tc.tile_pool(name="const", bufs=1) as cpool, \
             tc.tile_pool(name="pers", bufs=1) as pers, \
             tc.tile_pool(name="xp", bufs=4) as xp, \
             tc.tile_pool(name="wqp", bufs=4) as wqp, \
             tc.tile_pool(name="wkvp", bufs=4) as wkvp, \
             tc.tile_pool(name="sqp", bufs=3) as sqp, \
             tc.tile_pool(name="srp", bufs=3) as srp, \
             tc.tile_pool(name="rsp", bufs=3) as rsp, \
             tc.tile_pool(name="tp", bufs=3) as tp, \
             tc.tile_pool(name="twp", bufs=3) as twp, \
             tc.tile_pool(name="mp", bufs=3) as mp, \
             tc.tile_pool(name="probp", bufs=5) as probp, \
             tc.tile_pool(name="recp", bufs=2) as recp, \
             tc.tile_pool(name="pp2", bufs=3) as pp2, \
             tc.tile_pool(name="odp", bufs=4) as odp, \
             tc.tile_pool(name="psS", bufs=3, space="PSUM") as psS, \
             tc.tile_pool(name="psO", bufs=1, space="PSUM") as psO:

            holder = {}

            def _outproj(b, o_sb_t, wot_t, act_copy):
                for half in range(4):
                    ps_out = psS.tile([128, 1024], F32, tag="ps",
                                      name=f"po{b}_{half}")
                    for h in range(HQC):
                        for hc in range(2):
                            nc.tensor.matmul(
                                ps_out[:, hc * 512:(hc + 1) * 512],
                                o_sb_t[:, b * 512 + h * D:
                                       b * 512 + (h + 1) * D],
                                wot_t[:, h * HID + half * 1024 + hc * 512:
                                      h * HID + half * 1024 + (hc + 1) * 512],
                                start=(h == 0), stop=(h == HQC - 1))
                    od = odp.tile([128, 1024], BF16, tag="od")
                    if act_copy:
                        nc.scalar.activation(od[:, :], ps_out[:, :],
                                             mybir.ActivationFunctionType.Copy)
                    else:
                        nc.vector.tensor_copy(od[:, :], ps_out[:, :])
                    nc.sync.dma_start(
                        out=out_d[b * S:(b + 1) * S,
                                  half * 1024:(half + 1) * 1024],
                        in_=od[:, :])

            def body(_it, first=True):
                # ---- constants / tables (Pool SWDGE queue, small) ----
                cbt = cpool.tile([128, 1156], BF16, tag="cb")
                nc.gpsimd.dma_start(out=cbt[:, :], in_=cb_d[:, :])
                cft = cpool.tile([128, 6], F32, tag="cf")
                nc.gpsimd.dma_start(out=cft[:, :], in_=cf_d[:, :])
                sct = cpool.tile([128, 4 * BS], BF16, tag="sct")
                nc.gpsimd.dma_start(out=sct[:, :], in_=sct_d[:, :])
                allones = cbt[:, 0:128]
                negrow = cbt[0:1, 640:1152]
                wqA, wqB = cbt[:, 1152:1153], cbt[:, 1153:1154]
                wkA, wkB = cbt[:, 1154:1155], cbt[:, 1155:1156]
                epsq, epsk = cft[:, 4:5], cft[:, 5:6]
                sin2 = sct[:, 0:2 * BS]
                cos2 = sct[:, 2 * BS:4 * BS]
                atl1 = cpool.tile([128, 1], F32, tag="atl1")
                atl2 = cpool.tile([128, 1], F32, tag="atl2")

                # ---- phase A: Q,K,V projections in one streamed pass ----
                ps_kv = psS.tile([128, 1024], F32, tag="ps", name="ps_kv")
                ps_q01 = psS.tile([128, 1024], F32, tag="ps", name="ps_q01")
                ps_q23 = psS.tile([128, 1024], F32, tag="ps", name="ps_q23")
                ps_k = ps_kv[:, 0:512]
                ps_v = ps_kv[:, 512:1024]
                ps_qh = [ps_q01[:, 0:512], ps_q01[:, 512:1024],
                         ps_q23[:, 0:512], ps_q23[:, 512:1024]]
                GRP = 4
                xtiles = []
                for g in range(NDCH // GRP):
                    if g == 1:
                        # pre-load the Sqrt act table while ACT is idle
                        # (Square is present in every table set)
                        nc.scalar.activation(atl1[:, :], cft[:, 4:5], AF.Sqrt)
                    xtile = xp.tile([128, GRP * BS], BF16, tag="xt")
                    nc.sync.dma_start(out=xtile[:, :],
                                      in_=xT_v[:, g * GRP:(g + 1) * GRP, :])
                    xtiles.append(xtile)
                    wq = wqp.tile([128, GRP * EC], BF16, tag="wq")
                    nc.scalar.dma_start(out=wq[:, :],
                                        in_=wqT_v[:, g * GRP:(g + 1) * GRP, :])
                    wkv = wkvp.tile([128, GRP * 256], BF16, tag="wkv")
                    nc.scalar.dma_start(out=wkv[:, :],
                                        in_=wkvT_v[:, g * GRP:(g + 1) * GRP, :])
                    for j in range(GRP):
                        dch = g * GRP + j
                        st = dch == 0
                        sp = dch == NDCH - 1
                        xa = xtile[:, j * BS:(j + 1) * BS]
                        nc.tensor.matmul(ps_k, wkv[:, j * 256:j * 256 + D],
                                         xa, start=st, stop=sp)
                        nc.tensor.matmul(ps_v,
                                         wkv[:, j * 256 + D:(j + 1) * 256],
                                         xa, start=st, stop=sp)
                        for h in range(HQC):
                            nc.tensor.matmul(
                                ps_qh[h],
                                wq[:, j * EC + h * D:j * EC + (h + 1) * D],
                                xa, start=st, stop=sp)

                # resident loads behind phase A in their queues
                # gate the resident cache loads on a late x tile so their
                # transfers run after the phase-A streams, not against them
                kTt = pers.tile([128, total], BF16, tag="kT")
                nc.gpsimd.tensor_copy(kTt[0:1, 0:1], xtiles[6][0:1, 0:1])
                nc.gpsimd.dma_start(out=kTt[:, :], in_=kT_d[:, :])
                vCt = pers.tile([128, total], BF16, tag="vC")
                nc.gpsimd.tensor_copy(vCt[0:1, 0:1], xtiles[6][0:1, 0:1])
                nc.gpsimd.dma_start(out=vCt[:, :], in_=vC_d[:, :])

                # ---- norms + rope; k/v last so q-only work can start ----
                q_sb = pers.tile([128, HQC * BS], BF16, tag="q_sb")
                k_sb = pers.tile([128, BS], BF16, tag="k_sb")
                v_sb = pers.tile([128, BS], BF16, tag="v_sb")
                vt = pers.tile([128, BS], BF16, tag="vt")

                # psum readers first (frees phase-A accumulators for reuse).
                # Copies and rotate-halves run on Pool, squares/sqrt on ACT,
                # the rest on DVE, swaps on the SP hwdge queue - the serial
                # norm chain is spread across four engines.
                # GPSIMD cannot read PSUM on hw; ACT Copy is table-free.
                # Interleave square + copy per head group so each group's
                # rope math starts as early as possible.
                srcs = [ps_q01[:, :], ps_q23[:, :], ps_k]
                sqs, tsbs, tsws = [], [], []
                for i in range(3):
                    w = 1024 if i < 2 else 512
                    sq = sqp.tile([128, w], BF16, tag="sq", name=f"sq{i}")
                    nc.scalar.activation(sq[:, :], srcs[i], AF.Square)
                    sqs.append(sq)
                    t_sb = tp.tile([128, w], BF16, tag="t", name=f"t{i}")
                    nc.scalar.activation(t_sb[:, :], srcs[i], AF.Copy)
                    tsbs.append(t_sb)
                    tsw = twp.tile([128, w], BF16, tag="tw", name=f"tw{i}")
                    nc.sync.dma_start(out=tsw[0:64, :], in_=tsbs[i][64:128, :])
                    nc.sync.dma_start(out=tsw[64:128, :], in_=tsbs[i][0:64, :])
                    tsws.append(tsw)
                sq01, sq23, sqk = sqs
                nc.vector.tensor_copy(v_sb[:, :], ps_v)
                for b in range(B):
                    nc.sync.dma_start_transpose(
                        vt[:, b * S:(b + 1) * S], v_sb[:, b * S:(b + 1) * S])

                # sums (replicated across partitions via all-ones stationary)
                ss01 = psS.tile([128, 1024], F32, tag="ps", name="ss01")
                for hc in range(2):
                    nc.tensor.matmul(ss01[:, hc * 512:(hc + 1) * 512],
                                     allones, sq01[:, hc * 512:(hc + 1) * 512],
                                     start=True, stop=True)
                ss23 = psS.tile([128, 1024], F32, tag="ps", name="ss23")
                for hc in range(2):
                    nc.tensor.matmul(ss23[:, hc * 512:(hc + 1) * 512],
                                     allones, sq23[:, hc * 512:(hc + 1) * 512],
                                     start=True, stop=True)
                ssk = psS.tile([128, 512], F32, tag="ps", name="ssk")
                nc.tensor.matmul(ssk[:, :], allones, sqk[:, :],
                                 start=True, stop=True)

                # rstd (f32): q: 1/sqrt(ss + D*eps) (folds in 1/sqrt(D));
                # k: 1/sqrt(ss/D + eps)
                sr01 = srp.tile([128, 1024], BF16, tag="rs", name="sr01")
                nc.scalar.activation(sr01[:, :], ss01[:, :], AF.Sqrt,
                                     bias=epsq, scale=1.0)
                sr23 = srp.tile([128, 1024], BF16, tag="rs", name="sr23")
                nc.scalar.activation(sr23[:, :], ss23[:, :], AF.Sqrt,
                                     bias=epsq, scale=1.0)
                srk = srp.tile([128, 512], BF16, tag="rs", name="srk")
                nc.scalar.activation(srk[:, :], ssk[:, :], AF.Sqrt,
                                     bias=epsk, scale=1.0 / D)
                # absorb the Sqrt->Exp table switch while ACT is idle
                nc.scalar.activation(atl2[:, :], cft[:, 4:5], AF.Exp)
                # Wo loads issued here so their transfers fill the norm-phase
                # DMA idle window instead of competing with phase A streams
                wot = pers.tile([128, 4 * HID], BF16, tag="wo")
                for h in range(HQC):
                    nc.scalar.dma_start(out=wot[:, h * HID:(h + 1) * HID],
                                        in_=woT_v[:, h, :])

                dsts = [q_sb[:, 0:1024], q_sb[:, 1024:2048], k_sb[:, :]]
                rs = []
                for i in range(3):
                    w = 1024 if i < 2 else 512
                    wA, wB = (wqA, wqB) if i < 2 else (wkA, wkB)
                    sr = [sr01, sr23, srk][i]
                    m1 = mp.tile([128, w], BF16, tag="m", name=f"m1_{i}")
                    nc.vector.scalar_tensor_tensor(
                        m1[:, :], tsbs[i][:, :], wA, cos2[:, 0:w],
                        op0=OP.mult, op1=OP.mult)
                    m2 = mp.tile([128, w], BF16, tag="m", name=f"m2_{i}")
                    nc.vector.scalar_tensor_tensor(
                        m2[:, :], tsws[i][:, :], wB, sin2[:, 0:w],
                        op0=OP.mult, op1=OP.mult)
                    r = rsp.tile([128, w], BF16, tag="rs", name=f"r{i}")
                    with nc.allow_low_precision(reason="bf16 rstd"):
                        nc.vector.reciprocal(r[:, :], sr[:, :])
                    rs.append(r)
                    rt = mp.tile([128, w], BF16, tag="m", name=f"rt{i}")
                    nc.vector.tensor_sub(rt[0:64, :], m1[0:64, :], m2[0:64, :])
                    nc.vector.tensor_add(rt[64:128, :], m1[64:128, :],
                                         m2[64:128, :])
                    nc.vector.tensor_mul(dsts[i], rt[:, :], r[:, :])

                q4 = q_sb.rearrange("p (h b s) -> p h b s", h=HQC, b=B)
                o_sb = pers.tile([128, B * 512], BF16, tag="o_sb")
                holder['o_sb'] = o_sb
                holder['wot'] = wot

                def outproj(b):
                    _outproj(b, o_sb, wot, act_copy=False)

                for bi, b in enumerate(border):
                    ncache = nch[b]
                    tail = lens[b] - (ncache - 1) * 128 if ncache > 0 else 0
                    cis = list(range(ncache + 1))
                    groups = [cis[i:i + 2] for i in range(0, len(cis), 2)]
                    ngr = len(groups)
                    # [0:512] = unnormalized o, [512:1024] = prob sums
                    ps_os = psO.tile([128, 1024], F32, tag="po",
                                     name=f"pos{b}")

                    def kchunk(ci, b=b, ncache=ncache):
                        if ci == ncache:
                            return k_sb[:, b * S:(b + 1) * S]
                        return kTt[:, offs[b] + ci * 128:offs[b] + (ci + 1) * 128]

                    def vchunk(ci, b=b, ncache=ncache):
                        if ci == ncache:
                            return vt[:, b * S:(b + 1) * S]
                        return vCt[:, offs[b] + ci * 128:offs[b] + (ci + 1) * 128]

                    pending = []
                    sumq = []
                    sst = {'open': False, 'left': ncache + 1}

                    def drain_sums(final, ps_os=ps_os):
                        # pre-reduce up to 4 prob slices on DVE, then one
                        # ones-matmul per quartet (quarter the PE sum cost).
                        # The final ragged quartet goes per-slice: a DVE add
                        # chain there would stall the PE queue right where
                        # the next batch's QK wants to start.
                        while len(sumq) >= 4 or (final and sumq):
                            take = sumq[:4]
                            del sumq[:4]
                            if final and len(sumq) == 0:
                                for si, s in enumerate(take):
                                    st = not sst['open']
                                    sst['open'] = True
                                    sst['left'] -= 1
                                    nc.tensor.matmul(
                                        ps_os[:, 512:1024], allones, s,
                                        start=st, stop=sst['left'] == 0)
                                return
                            if len(take) == 1:
                                mv = take[0]
                            else:
                                t1 = pp2.tile([128, 512], BF16, tag="pp2")
                                nc.vector.tensor_add(t1[:, :], take[0],
                                                     take[1])
                                mv = t1[:, :]
                                if len(take) >= 3:
                                    if len(take) == 4:
                                        t2 = pp2.tile([128, 512], BF16,
                                                      tag="pp2")
                                        nc.vector.tensor_add(t2[:, :],
                                                             take[2], take[3])
                                        m2 = t2[:, :]
                                    else:
                                        m2 = take[2]
                                    t3 = pp2.tile([128, 512], BF16, tag="pp2")
                                    nc.vector.tensor_add(t3[:, :], mv, m2)
                                    mv = t3[:, :]
                            st = not sst['open']
                            sst['open'] = True
                            sst['left'] -= len(take)
                            sp = sst['left'] == 0
                            nc.tensor.matmul(ps_os[:, 512:1024], allones, mv,
                                             start=st, stop=sp)

                    def flush(gi_, prob_, width_, ps_os=ps_os, ngr=ngr,
                              groups=groups):
                        first = gi_ == 0
                        last = gi_ == ngr - 1
                        nk = width_ // 512
                        for k in range(nk):
                            ci = groups[gi_][k]
                            pr = prob_[:, k * 512:(k + 1) * 512]
                            st = first and k == 0
                            sp = last and k == nk - 1
                            nc.tensor.matmul(ps_os[:, 0:512], vchunk(ci), pr,
                                             start=st, stop=sp)
                            sumq.append(pr)
                        drain_sums(False)

                    for gi, grp in enumerate(groups):
                        width = 512 * len(grp)
                        ps_s = psS.tile([128, 1024], F32, tag="ps",
                                        name=f"s{b}_{gi}")
                        for k, ci in enumerate(grp):
                            masked = (ci < ncache and ci == ncache - 1
                                      and tail < 128)
                            if bi == 0 and gi < 3 and ci < ncache - 1:
                                # first chunks: split by head pair so PE can
                                # start before all 4 heads are roped
                                for hp in range(2):
                                    nc.tensor.matmul(
                                        ps_s[:, k * 512 + hp * 256:
                                             k * 512 + (hp + 1) * 256],
                                        kchunk(ci),
                                        q4[:, 2 * hp:2 * hp + 2, b, :],
                                        start=True, stop=True)
                                continue
                            nc.tensor.matmul(ps_s[:, k * 512:(k + 1) * 512],
                                             kchunk(ci), q4[:, :, b, :],
                                             start=True, stop=not masked)
                            if masked:
                                nc.tensor.matmul(
                                    ps_s[:, k * 512:(k + 1) * 512],
                                    cbt[0:1, 128 + b * 128:128 + (b + 1) * 128],
                                    negrow, start=False, stop=True)
                        prob = probp.tile([128, 1024], BF16, tag="prob")
                        nc.scalar.activation(prob[:, 0:width],
                                             ps_s[:, 0:width], AF.Exp)
                        pending.append((gi, prob, width))
                        if len(pending) > 2:
                            flush(*pending.pop(0))
                        if bi > 0 and gi == min(3, ngr - 1):
                            outproj(border[bi - 1])
                    while pending:
                        flush(*pending.pop(0))
                    drain_sums(True)

                    recb = recp.tile([128, 512], F32, tag="rec")
                    nc.vector.reciprocal(recb[:, :], ps_os[:, 512:1024])
                    nc.vector.tensor_mul(o_sb[:, b * 512:(b + 1) * 512],
                                         ps_os[:, 0:512], recb[:, :])
                outproj(border[-1])

            if reps == 1:
                body(0)
            else:
                with tc.For_i(0, reps, 1,
                              hint_engines=(mybir.EngineType.PE,
                                            mybir.EngineType.Activation,
                                            mybir.EngineType.Pool,
                                            mybir.EngineType.DVE,
                                            mybir.EngineType.SP)) as it:
                    body(it)

    nc.compile()
    return nc


def _get_nc(lens, pads, offs, total, reps=1, phases=3):
    key = (tuple(lens), total, reps)
    if key not in _CACHE:
        _CACHE[key] = _build_nc(lens, pads, offs, total, reps)
    return _CACHE[key]


def kernel(x, Wq, Wk, Wv, Wo, q_norm_w, k_norm_w, k_cache, v_cache,
           block_table, cache_seqlens):
    from concourse.bass_utils import run_bass_kernel_spmd

    in_maps, lens, pads, offs, total = _prep_host(
        x, Wq, Wk, Wv, Wo, q_norm_w, k_norm_w, k_cache, v_cache,
        block_table, cache_seqlens)
    nc = _get_nc(lens, pads, offs, total, reps=1)
    res = run_bass_kernel_spmd(nc, in_maps, core_ids=list(range(N_CORES)))
    partials = np.stack([np.asarray(r["out"], np.float32)
                         for r in res.results], 0)
    out = np.sum(partials, axis=0, dtype=np.float64).astype(np.float32)
    return out.reshape(B, S, HID)



# revision 24
# speedup vs baseline: 1.2466x; 1.2466x over previous
"""DFlashAttention (paged KV cache decode-attention block) on 8 Trainium2
NeuronCores.

Sharding: tensor-parallel over heads. Each core owns HQ/8 = 4 query heads and
HK/8 = 1 KV head (GQA group). Wq/Wk/Wv row-sharded, Wo column-sharded; each
core produces a partial output [B*S, HID] (stored bf16) which is summed on
the host.

v3 schedule (bf16 data path, PE-saturating):
  - All inputs are pre-laid-out on the host as contiguous [128, N] SBUF
    images so every DMA moves multi-KB per-partition lines at full HBM
    bandwidth.
  - One deadline-ordered DMA stream on the SP queue feeds everything into
    persistent tiles (no write-after-read hazards, so the queue never
    stalls): consts, interleaved x/Wq pieces, sin-cos, Wk, Wv, then the
    per-batch KV-cache parts and Wo ordered by first use.
  - Phase A: Q projection (chunk-major, heads inner) -> K stream -> V
    stream.  The q/k norm+rope chains (ACT/DVE/swap) and their sum matmuls
    are interleaved under the K/V streams, so attention starts with no
    bridge gap.  Sqrt/Exp table loads are prefetched into ACT-idle windows.
  - Scores computed transposed [l_chunk, (h,s)] so PV consumes probs
    directly; softmax sums via all-ones stationary matmuls fed by an eager
    8-deep DVE pre-reduction forest.  Cache-tail masking via per-partition
    Exp bias.
  - Output projection for batch b is emitted inside batch b+1's attention
    so its matmuls fill PE gaps; batches processed in descending length.
"""

import sys

sys.path.insert(0, "/opt/trn_rl_repo")

import numpy as np

B, S, HID = 4, 128, 4096
D, HQ, HK = 128, 32, 8
PAGES, PSIZE, NPP = 64, 256, 16
THETA = 10000.0
EPS = 1e-6
N_CORES = 8
HQC = HQ // N_CORES  # 4 query heads per core
EC = HQC * D         # 512 output-proj contraction per core
BS = B * S           # 512 tokens
NDCH = HID // 128    # 32 contraction chunks for projections

_CACHE = {}


def _img(mT):
    """[HID-like rows, C cols] -> contiguous SBUF image [128, (chunk, C)]."""
    r, c = mT.shape
    return np.ascontiguousarray(
        mT.reshape(r // 128, 128, c).transpose(1, 0, 2).reshape(128, -1))


def _prep_host(x, Wq, Wk, Wv, Wo, q_norm_w, k_norm_w, k_cache, v_cache,
               block_table, cache_seqlens):
    import ml_dtypes
    BF = ml_dtypes.bfloat16
    f32 = np.float32

    xT = np.asarray(x, f32).reshape(BS, HID).T          # [HID, BS]
    xS = _img(xT).astype(BF)                            # [128, 32*512]

    lens = [int(v) for v in np.asarray(cache_seqlens)]
    pads = [(l + 127) // 128 * 128 for l in lens]
    offs = [0] * B
    for b in range(1, B):
        offs[b] = offs[b - 1] + pads[b - 1]
    total = max(sum(pads), 128)

    bt = np.asarray(block_table)
    kg = np.asarray(k_cache, f32)[bt].reshape(B, NPP * PSIZE, HK, D)
    vg = np.asarray(v_cache, f32)[bt].reshape(B, NPP * PSIZE, HK, D)

    # RoPE sin/cos evaluated on host in f64 on the reference's fp32 freqs.
    pos = np.asarray(cache_seqlens, np.float64)[:, None] + np.arange(S)[None, :]
    inv = 1.0 / (THETA ** (np.arange(0, D, 2, dtype=np.float64) / D))
    freqs32 = (pos.astype(f32)[:, :, None]
               * inv.astype(f32)[None, None, :]).astype(f32)
    fr = np.float64(freqs32)
    sin_h = np.sin(fr).reshape(BS, 64).T            # [64, BS]
    cos_h = np.cos(fr).reshape(BS, 64).T
    # sign-folded: rope combine becomes a single add over all partitions
    sin2 = np.concatenate([-sin_h, sin_h], 0)       # [128, BS]
    cos2 = np.concatenate([cos_h, cos_h], 0)
    wq_ = np.asarray(q_norm_w, f32).reshape(D)
    wk_ = np.asarray(k_norm_w, f32).reshape(D)
    # norm weights premultiplied into the tables (tensor_tensor on DVE gets
    # the 2x bf16 mode; scalar_tensor_tensor does not)
    sinq = sin2 * np.roll(wq_, 64)[:, None]
    cosq = cos2 * wq_[:, None]
    sink = sin2 * np.roll(wk_, 64)[:, None]
    cosk = cos2 * wk_[:, None]
    # q tables doubled along columns so one op covers a 2-head pair tile
    sct = np.ascontiguousarray(np.concatenate(
        [sinq, sinq, cosq, cosq, sink, cosk], 1)).astype(BF)  # [128, 6*BS]

    # f32 per-partition scalars: epsq epsk + 4 mask-bias cols
    cols = [np.full(D, D * EPS, f32), np.full(D, EPS, f32)]
    for b in range(B):
        tail = lens[b] - (pads[b] // 128 - 1) * 128 if pads[b] > 0 else 128
        mb = np.zeros(D, f32)
        mb[tail:] = -1e30
        cols.append(mb)
    cf = np.ascontiguousarray(np.stack(cols, 1))    # [128, 6]

    # bf16 consts: all-ones stationary for partition-broadcast sums
    cb = np.ones((128, 128), f32).astype(BF)

    Wq_ = np.asarray(Wq, f32)
    Wk_ = np.asarray(Wk, f32)
    Wv_ = np.asarray(Wv, f32)
    Wo_ = np.asarray(Wo, f32)

    in_maps = []
    for c in range(N_CORES):
        # Wq image, head-pair-major: [128, (pair, chunk, head-in-pair, 128)]
        # so each pair's stream is a contiguous 2 MB block
        wqI = _img(Wq_[c * EC:(c + 1) * EC, :].T)   # [128, (chunk, h, 128)]
        wqI = wqI.reshape(128, NDCH, 2, 2 * D)
        wqS = np.ascontiguousarray(
            wqI.transpose(0, 2, 1, 3).reshape(128, NDCH * EC)).astype(BF)
        wkS = _img(Wk_[c * D:(c + 1) * D, :].T).astype(BF)
        wvS = _img(Wv_[c * D:(c + 1) * D, :].T).astype(BF)
        # Wo image, half-major: [128, (half, h, 1024)]
        woT = np.ascontiguousarray(Wo_[:, c * EC:(c + 1) * EC].T)  # [EC, HID]
        woS = np.ascontiguousarray(
            woT.reshape(HQC, 128, 4, 1024).transpose(1, 2, 0, 3)
            .reshape(128, 4 * HID)).astype(BF)
        kT = np.zeros((128, total), f32)
        vCf = np.zeros((total, 128), f32)
        for b in range(B):
            nb, ob = lens[b], offs[b]
            if nb > 0:
                kT[:, ob:ob + nb] = kg[b, :nb, c, :].T
                vCf[ob:ob + nb, :] = vg[b, :nb, c, :]
        vP = np.ascontiguousarray(
            vCf.reshape(total // 128, 128, 128).transpose(1, 0, 2)
            .reshape(128, total))
        in_maps.append(dict(
            xS=xS, wqS=wqS, wkS=wkS, wvS=wvS, woS=woS,
            kT=np.ascontiguousarray(kT).astype(BF), vC=vP.astype(BF),
            sct=sct, cf=cf, cb=cb,
        ))
    return in_maps, lens, pads, offs, total


def _build_nc(lens, pads, offs, total, reps=1):
    import concourse.mybir as mybir
    import concourse.tile as tile
    from concourse import bacc

    F32 = mybir.dt.float32
    BF16 = mybir.dt.bfloat16
    AF = mybir.ActivationFunctionType
    OP = mybir.AluOpType

    nc = bacc.Bacc("TRN2", target_bir_lowering=False, debug=False,
                   num_devices=N_CORES)

    xS_d = nc.dram_tensor("xS", [128, NDCH * BS], BF16, kind="ExternalInput")
    wqS_d = nc.dram_tensor("wqS", [128, NDCH * EC], BF16, kind="ExternalInput")
    wkS_d = nc.dram_tensor("wkS", [128, NDCH * D], BF16, kind="ExternalInput")
    wvS_d = nc.dram_tensor("wvS", [128, NDCH * D], BF16, kind="ExternalInput")
    woS_d = nc.dram_tensor("woS", [128, 4 * HID], BF16, kind="ExternalInput")
    kT_d = nc.dram_tensor("kT", [128, total], BF16, kind="ExternalInput")
    vC_d = nc.dram_tensor("vC", [128, total], BF16, kind="ExternalInput")
    sct_d = nc.dram_tensor("sct", [128, 6 * BS], BF16, kind="ExternalInput")
    cf_d = nc.dram_tensor("cf", [128, 6], F32, kind="ExternalInput")
    cb_d = nc.dram_tensor("cb", [128, 128], BF16, kind="ExternalInput")
    out_d = nc.dram_tensor("out", [BS, HID], BF16, kind="ExternalOutput")

    nch = [pads[b] // 128 for b in range(B)]
    border = sorted(range(B), key=lambda b: -nch[b])

    with tile.TileContext(nc) as tc:
        with tc.tile_pool(name="const", bufs=1) as cpool, \
             tc.tile_pool(name="pers", bufs=1) as pers, \
             tc.tile_pool(name="wqp", bufs=4) as wqp, \
             tc.tile_pool(name="sqp", bufs=2) as sqp, \
             tc.tile_pool(name="srp", bufs=1) as srp, \
             tc.tile_pool(name="rsp", bufs=2) as rsp, \
             tc.tile_pool(name="tp", bufs=2) as tp, \
             tc.tile_pool(name="twp", bufs=2) as twp, \
             tc.tile_pool(name="mp", bufs=3) as mp, \
             tc.tile_pool(name="probp", bufs=4) as probp, \
             tc.tile_pool(name="recp", bufs=1) as recp, \
             tc.tile_pool(name="pp2", bufs=5) as pp2, \
             tc.tile_pool(name="odp", bufs=3) as odp, \
             tc.tile_pool(name="psS", bufs=3, space="PSUM") as psS, \
             tc.tile_pool(name="psO", bufs=1, space="PSUM") as psO:

            def _outproj(b, o_sb_t, wot_t, act_copy=False):
                for half in range(4):
                    ps_out = psS.tile([128, 1024], F32, tag="ps",
                                      name=f"po{b}_{half}")
                    for h in range(HQC):
                        for hc in range(2):
                            nc.tensor.matmul(
                                ps_out[:, hc * 512:(hc + 1) * 512],
                                o_sb_t[:, b * 512 + h * D:
                                       b * 512 + (h + 1) * D],
                                wot_t[:, half * HID + h * 1024 + hc * 512:
                                      half * HID + h * 1024 + (hc + 1) * 512],
                                start=(h == 0), stop=(h == HQC - 1))
                    od = odp.tile([128, 1024], BF16, tag="od")
                    if act_copy:
                        # split across ACT+DVE so the drain tail is short
                        nc.vector.tensor_copy(od[:, 0:512], ps_out[:, 0:512])
                        nc.sync.dma_start(
                            out=out_d[b * S:(b + 1) * S,
                                      half * 1024:half * 1024 + 512],
                            in_=od[:, 0:512])
                        nc.scalar.activation(
                            od[:, 512:1024], ps_out[:, 512:1024],
                            mybir.ActivationFunctionType.Copy)
                        nc.sync.dma_start(
                            out=out_d[b * S:(b + 1) * S,
                                      half * 1024 + 512:(half + 1) * 1024],
                            in_=od[:, 512:1024])
                    else:
                        nc.vector.tensor_copy(od[:, :], ps_out[:, :])
                        nc.sync.dma_start(
                            out=out_d[b * S:(b + 1) * S,
                                      half * 1024:(half + 1) * 1024],
                            in_=od[:, :])

            def body(_it, first=True):
                # ---- persistent tiles ----
                cbt = cpool.tile([128, 128], BF16, tag="cb")
                cft = cpool.tile([128, 6], F32, tag="cf")
                kTt = pers.tile([128, total], BF16, tag="kT")
                vCt = pers.tile([128, total], BF16, tag="vC")
                sct = cpool.tile([128, 6 * BS], BF16, tag="sct")
                xs = pers.tile([128, NDCH * BS], BF16, tag="xs")
                wkt = pers.tile([128, NDCH * D], BF16, tag="wk")
                wvt = pers.tile([128, NDCH * D], BF16, tag="wv")
                wot = pers.tile([128, 4 * HID], BF16, tag="wo")
                q_sb = pers.tile([128, HQC * BS], BF16, tag="q_sb")
                k_sb = pers.tile([128, BS], BF16, tag="k_sb")
                v_sb = pers.tile([128, BS], BF16, tag="v_sb")
                vt = pers.tile([128, BS], BF16, tag="vt")
                o_sb = pers.tile([128, B * 512], BF16, tag="o_sb")

                # ---- start of the ordered DMA stream (SP queue; strict
                # FIFO into persistent tiles = deadline-ordered transfers,
                # the queue never blocks on tile reuse) ----
                allones = cbt[:, 0:128]
                epsq, epsk = cft[:, 0:1], cft[:, 1:2]
                maskb = [cft[:, 2 + b:3 + b] for b in range(B)]
                sinq = sct[:, 0:2 * BS]
                cosq = sct[:, 2 * BS:4 * BS]
                sink = sct[:, 4 * BS:5 * BS]
                cosk = sct[:, 5 * BS:6 * BS]
                atl1 = cpool.tile([128, 1], F32, tag="atl1")
                atl2 = cpool.tile([128, 1], F32, tag="atl2")

                # ---- phase A part 1: Q projection in two head-pair passes
                # (pair 0's norm chain runs under pair 1's matmul stream) ----
                ps_q01 = psS.tile([128, 1024], F32, tag="ps", name="ps_q01")
                ps_q23 = psS.tile([128, 1024], F32, tag="ps", name="ps_q23")
                ps_qp = [ps_q01, ps_q23]
                sqs, tsbs, tsws, rqs = [], [], [], []
                sss = [psO.tile([128, 1024], F32, tag="po", name=f"ss{i}")
                       for i in range(2)]

                def qchain_pre(i):
                    # ACT part of pair i's norm chain (after ps stops)
                    sq = sqp.tile([128, 1024], BF16, tag="sq", name=f"sq{i}")
                    nc.scalar.activation(sq[:, :], ps_qp[i][:, :], AF.Square)
                    sqs.append(sq)

                def qsum(i):
                    # PE sum + ACT rstd for pair i (emit inside a PE stream)
                    for hc in range(2):
                        nc.tensor.matmul(
                            sss[i][:, hc * 512:(hc + 1) * 512], allones,
                            sqs[i][:, hc * 512:(hc + 1) * 512],
                            start=True, stop=True)
                    sr = srp.tile([128, 1024], BF16, tag="sr", name=f"sr{i}")
                    nc.scalar.activation(sr[:, :], sss[i][:, :], AF.Sqrt,
                                         bias=epsq, scale=1.0)
                    rq = rsp.tile([128, 1024], BF16, tag="rs", name=f"rq{i}")
                    with nc.allow_low_precision(reason="bf16 rstd"):
                        nc.vector.reciprocal(rq[:, :], sr[:, :])
                    rqs.append(rq)

                def qrope(i):
                    # DVE part: rope mults straight off PSUM (the rotate-half
                    # reads PSUM at a crossed base partition, which the
                    # verifier allows for non-SBUF inputs) + rstd scale
                    m1 = mp.tile([128, 1024], BF16, tag="m", name=f"m1_{i}")
                    nc.vector.tensor_mul(m1[:, :], ps_qp[i][:, :],
                                         cosq[:, 0:1024])
                    m2 = mp.tile([128, 1024], BF16, tag="m", name=f"m2_{i}")
                    nc.vector.tensor_mul(m2[0:64, :], ps_qp[i][64:128, :],
                                         sinq[0:64, 0:1024])
                    nc.vector.tensor_mul(m2[64:128, :], ps_qp[i][0:64, :],
                                         sinq[64:128, 0:1024])
                    rt = mp.tile([128, 1024], BF16, tag="m", name=f"rt{i}")
                    nc.vector.tensor_add(rt[:, :], m1[:, :], m2[:, :])
                    nc.vector.tensor_mul(q_sb[:, i * 1024:(i + 1) * 1024],
                                         rt[:, :], rqs[i][:, :])

                GRP = 4
                NP = NDCH // GRP
                for pair in range(2):
                    for g in range(NP):
                        if pair == 0:
                            if g == 0:
                                nc.sync.dma_start(out=xs[:, 0:BS],
                                                  in_=xS_d[:, 0:BS])
                            wq = wqp.tile([128, GRP * 2 * D], BF16, tag="wq")
                            if g == 0:
                                nc.sync.dma_start(out=wq[:, 0:256],
                                                  in_=wqS_d[:, 0:256])
                                nc.sync.dma_start(out=xs[:, BS:2 * BS],
                                                  in_=xS_d[:, BS:2 * BS])
                                nc.sync.dma_start(out=wq[:, 256:1024],
                                                  in_=wqS_d[:, 256:1024])
                            else:
                                nc.sync.dma_start(
                                    out=wq[:, :],
                                    in_=wqS_d[:, g * 1024:(g + 1) * 1024])
                            if g == 0:
                                nc.sync.dma_start(out=cbt[:, :], in_=cb_d[:, :])
                                nc.sync.dma_start(out=cft[:, :], in_=cf_d[:, :])
                                nc.sync.dma_start(
                                    out=xs[:, 2 * BS:GRP * BS],
                                    in_=xS_d[:, 2 * BS:GRP * BS])
                            else:
                                nc.sync.dma_start(
                                    out=xs[:, g * GRP * BS:(g + 1) * GRP * BS],
                                    in_=xS_d[:, g * GRP * BS:(g + 1) * GRP * BS])
                        else:
                            wq = wqp.tile([128, GRP * 2 * D], BF16, tag="wq")
                            nc.sync.dma_start(
                                out=wq[:, :],
                                in_=wqS_d[:, NDCH * 2 * D + g * 1024:
                                          NDCH * 2 * D + (g + 1) * 1024])
                            if g == 2:
                                # sin/cos lands just before the first rope use
                                nc.sync.dma_start(out=sct[:, :],
                                                  in_=sct_d[:, :])
                        if pair == 0 and g == 3:
                            # prefetch the Sqrt act table while ACT is idle
                            nc.scalar.activation(atl1[:, :], cft[:, 0:1],
                                                 AF.Sqrt)
                        if pair == 1 and g == 1:
                            qsum(0)
                        for j in range(GRP):
                            dch = g * GRP + j
                            st = dch == 0
                            sp = dch == NDCH - 1
                            xa = xs[:, dch * BS:(dch + 1) * BS]
                            for hp in range(2):
                                nc.tensor.matmul(
                                    ps_qp[pair][:, hp * 512:(hp + 1) * 512],
                                    wq[:, j * 2 * D + hp * D:
                                       j * 2 * D + (hp + 1) * D],
                                    xa, start=st, stop=sp)
                    qchain_pre(pair)
                    if pair == 1:
                        qrope(0)

                # rest of the input stream, in deadline order
                half_kc = NDCH * D // 2
                for i in range(2):
                    nc.sync.dma_start(
                        out=wkt[:, i * half_kc:(i + 1) * half_kc],
                        in_=wkS_d[:, i * half_kc:(i + 1) * half_kc])
                for i in range(2):
                    nc.sync.dma_start(
                        out=wvt[:, i * half_kc:(i + 1) * half_kc],
                        in_=wvS_d[:, i * half_kc:(i + 1) * half_kc])

                def ld_kT(b):
                    nc.sync.dma_start(
                        out=kTt[:, offs[b]:offs[b] + pads[b]],
                        in_=kT_d[:, offs[b]:offs[b] + pads[b]])

                def ld_vC(b):
                    nc.sync.dma_start(
                        out=vCt[:, offs[b]:offs[b] + pads[b]],
                        in_=vC_d[:, offs[b]:offs[b] + pads[b]])

                def ld_wo(i):
                    nc.sync.dma_start(
                        out=wot[:, i * HID:(i + 1) * HID],
                        in_=woS_d[:, i * HID:(i + 1) * HID])

                ld_kT(border[0])
                ld_vC(border[0])
                ld_kT(border[1])
                ld_wo(0)
                ld_wo(1)
                ld_vC(border[1])
                ld_wo(2)
                ld_wo(3)
                ld_kT(border[2])
                ld_vC(border[2])
                ld_kT(border[3])
                ld_vC(border[3])

                # ---- K stream; pair-1 q-chain work interleaves ----
                ps_k = psS.tile([128, 512], F32, tag="ps", name="ps_k")
                for j in range(NDCH):
                    nc.tensor.matmul(ps_k[:, :], wkt[:, j * D:(j + 1) * D],
                                     xs[:, j * BS:(j + 1) * BS],
                                     start=(j == 0), stop=(j == NDCH - 1))
                    if j == 2:
                        qsum(1)
                qrope(1)

                # k norm chain (ACT part) — runs as soon as ps_k stops
                sqk = sqp.tile([128, 512], BF16, tag="sq", name="sqk")
                nc.scalar.activation(sqk[:, :], ps_k[:, :], AF.Square)

                # V stream; ssk + k rope chain interleave under it
                ps_v = psS.tile([128, 512], F32, tag="ps", name="ps_v")
                ssk = psO.tile([128, 512], F32, tag="po", name="ssk")
                rk = rsp.tile([128, 512], BF16, tag="rs", name="rk")
                for j in range(NDCH):
                    nc.tensor.matmul(ps_v[:, :], wvt[:, j * D:(j + 1) * D],
                                     xs[:, j * BS:(j + 1) * BS],
                                     start=(j == 0), stop=(j == NDCH - 1))
                    if j == 3:
                        nc.tensor.matmul(ssk[:, :], allones, sqk[:, :],
                                         start=True, stop=True)
                        srk = srp.tile([128, 512], BF16, tag="sr", name="srk")
                        nc.scalar.activation(srk[:, :], ssk[:, :], AF.Sqrt,
                                             bias=epsk, scale=1.0 / D)
                        # prefetch Exp table: all Sqrt uses are done
                        nc.scalar.activation(atl2[:, :], cft[:, 0:1], AF.Exp)
                        with nc.allow_low_precision(reason="bf16 rstd"):
                            nc.vector.reciprocal(rk[:, :], srk[:, :])
                # v_sb + transposes first: they gate the attention PSUM
                # rotation and the fresh-V chunks
                nc.vector.tensor_copy(v_sb[:, :], ps_v[:, :])
                for b in range(B):
                    nc.sync.dma_start_transpose(
                        vt[:, b * S:(b + 1) * S], v_sb[:, b * S:(b + 1) * S])
                m1k = mp.tile([128, 512], BF16, tag="m", name="m1k")
                nc.vector.tensor_mul(m1k[:, :], ps_k[:, :], cosk[:, :])
                m2k = mp.tile([128, 512], BF16, tag="m", name="m2k")
                nc.vector.tensor_mul(m2k[0:64, :], ps_k[64:128, :],
                                     sink[0:64, :])
                nc.vector.tensor_mul(m2k[64:128, :], ps_k[0:64, :],
                                     sink[64:128, :])
                rtk = mp.tile([128, 512], BF16, tag="m", name="rtk")
                nc.vector.tensor_add(rtk[:, :], m1k[:, :], m2k[:, :])
                nc.vector.tensor_mul(k_sb[:, :], rtk[:, :], rk[:, :])

                q4 = q_sb.rearrange("p (h b s) -> p h b s", h=HQC, b=B)

                def outproj(b, act_copy=False):
                    _outproj(b, o_sb, wot, act_copy=act_copy)

                # ---- attention ----
                for bi, b in enumerate(border):
                    ncache = nch[b]
                    tail = lens[b] - (ncache - 1) * 128 if ncache > 0 else 0
                    cis = list(range(ncache + 1))
                    groups = [cis[i:i + 2] for i in range(0, len(cis), 2)]
                    ngr = len(groups)
                    # [0:512] = unnormalized o, [512:1024] = prob sums
                    ps_os = psO.tile([128, 1024], F32, tag="po",
                                     name=f"pos{b}")

                    def kchunk(ci, b=b, ncache=ncache):
                        if ci == ncache:
                            return k_sb[:, b * S:(b + 1) * S]
                        return kTt[:, offs[b] + ci * 128:offs[b] + (ci + 1) * 128]

                    def vchunk(ci, b=b, ncache=ncache):
                        if ci == ncache:
                            return vt[:, b * S:(b + 1) * S]
                        return vCt[:, offs[b] + ci * 128:offs[b] + (ci + 1) * 128]

                    pending = []
                    # eager binary reduction forest of prob slices; level-3
                    # roots (8 slices) are matmul'd into the sum as they form
                    forest = []
                    nslices = ncache + 1
                    # ones-matmul count: one per full 8-tree + one per set
                    # bit of the remainder (forest nodes left at the end)
                    total_roots = nslices // 8 + bin(nslices % 8).count("1")
                    sst = {'open': False, 'roots_left': total_roots}

                    def emit_root(ap, ps_os=ps_os):
                        st = not sst['open']
                        sst['open'] = True
                        sst['roots_left'] -= 1
                        nc.tensor.matmul(ps_os[:, 512:1024], allones, ap,
                                         start=st, stop=sst['roots_left'] == 0)

                    def push_prob(pr):
                        forest.append((0, pr))
                        while (len(forest) >= 2
                               and forest[-1][0] == forest[-2][0]):
                            l2, a2 = forest.pop()
                            l1, a1 = forest.pop()
                            t = pp2.tile([128, 512], BF16, tag="pp2")
                            nc.vector.tensor_add(t[:, :], a1, a2)
                            if l1 + 1 == 3:
                                emit_root(t[:, :])
                            else:
                                forest.append((l1 + 1, t[:, :]))

                    def drain_forest():
                        for _, ap in forest:
                            emit_root(ap)
                        forest.clear()

                    def flush(gi_, prob_, width_, ps_os=ps_os, ngr=ngr,
                              groups=groups, push=True):
                        first = gi_ == 0
                        last = gi_ == ngr - 1
                        nk = width_ // 512
                        for k in range(nk):
                            ci = groups[gi_][k]
                            pr = prob_[:, k * 512:(k + 1) * 512]
                            st = first and k == 0
                            sp = last and k == nk - 1
                            nc.tensor.matmul(ps_os[:, 0:512], vchunk(ci), pr,
                                             start=st, stop=sp)
                            if push:
                                push_prob(pr)

                    mci = ncache - 1 if (ncache > 0 and tail < 128) else -1
                    for gi, grp in enumerate(groups):
                        width = 512 * len(grp)
                        ps_s = psS.tile([128, 1024], F32, tag="ps",
                                        name=f"s{b}_{gi}")
                        for k, ci in enumerate(grp):
                            nc.tensor.matmul(ps_s[:, k * 512:(k + 1) * 512],
                                             kchunk(ci), q4[:, :, b, :],
                                             start=True, stop=True)
                        prob = probp.tile([128, 1024], BF16, tag="prob")
                        if mci in grp:
                            for k, ci in enumerate(grp):
                                if ci == mci:
                                    nc.scalar.activation(
                                        prob[:, k * 512:(k + 1) * 512],
                                        ps_s[:, k * 512:(k + 1) * 512],
                                        AF.Exp, bias=maskb[b], scale=1.0)
                                else:
                                    nc.scalar.activation(
                                        prob[:, k * 512:(k + 1) * 512],
                                        ps_s[:, k * 512:(k + 1) * 512],
                                        AF.Exp)
                        else:
                            nc.scalar.activation(prob[:, 0:width],
                                                 ps_s[:, 0:width], AF.Exp)
                        pending.append((gi, prob, width))
                        op_gi = min(5, ngr - 1)
                        if bi == B - 1 and gi >= ngr - 3:
                            pdepth = 1
                        elif bi > 0 and op_gi - 1 <= gi <= op_gi + 1:
                            pdepth = 2
                        else:
                            pdepth = 3
                        while len(pending) > pdepth:
                            flush(*pending.pop(0))
                        if bi > 0 and gi == op_gi:
                            outproj(border[bi - 1])
                    # push remaining probs + close the sums first, so the
                    # reciprocal overlaps the tail PV matmuls on the PE
                    for (gi_, prob_, width_) in pending:
                        for k in range(width_ // 512):
                            push_prob(prob_[:, k * 512:(k + 1) * 512])
                    drain_forest()
                    recb = recp.tile([128, 512], F32, tag="rec")
                    nc.vector.reciprocal(recb[:, :], ps_os[:, 512:1024])
                    while pending:
                        flush(*pending.pop(0), push=False)
                    nc.vector.tensor_mul(o_sb[:, b * 512:(b + 1) * 512],
                                         ps_os[:, 0:512], recb[:, :])
                outproj(border[-1], act_copy=True)

            if reps == 1:
                body(0)
            else:
                with tc.For_i(0, reps, 1,
                              hint_engines=(mybir.EngineType.PE,
                                            mybir.EngineType.Activation,
                                            mybir.EngineType.Pool,
                                            mybir.EngineType.DVE,
                                            mybir.EngineType.SP)) as it:
                    body(it)

    nc.compile()
    return nc


def _get_nc(lens, pads, offs, total, reps=1, phases=3):
    key = (tuple(lens), total, reps)
    if key not in _CACHE:
        _CACHE[key] = _build_nc(lens, pads, offs, total, reps)
    return _CACHE[key]


def kernel(x, Wq, Wk, Wv, Wo, q_norm_w, k_norm_w, k_cache, v_cache,
           block_table, cache_seqlens):
    from concourse.bass_utils import run_bass_kernel_spmd

    in_maps, lens, pads, offs, total = _prep_host(
        x, Wq, Wk, Wv, Wo, q_norm_w, k_norm_w, k_cache, v_cache,
        block_table, cache_seqlens)
    nc = _get_nc(lens, pads, offs, total, reps=1)
    res = run_bass_kernel_spmd(nc, in_maps, core_ids=list(range(N_CORES)))
    partials = np.stack([np.asarray(r["out"], np.float32)
                         for r in res.results], 0)
    out = np.sum(partials, axis=0, dtype=np.float64).astype(np.float32)
    return out.reshape(B, S, HID)
